# revision 40
# baseline (speedup 1.0000x reference)
"""Trainium2 Bass kernel for CSPFM-style pooled channel-attention broadcast.

Math (per batch b):
    d = max(x[b], spatial)                       # [C]
    e = mean(x[b], spatial)                      # [C]
    z = d outer d + e outer e                    # [C, C]  (symmetric!)
    y = softmax(z, axis=-1)
    f = alpha * (d @ y) + beta * (e @ y)         # [C]
    out[b, c, :, :] = f[c]

Key restructure vs the naive version: because z is symmetric,
    f[j] = sum_i g_i y[i,j]            with g = alpha*d + beta*e
         = e^{m_j} * sum_i w_i E[j,i]  with w_i = g_i e^{-m_i}/s_i,
           E[j,i] = exp(z[j,i] - m_j)  (the very softmax-numerator tiles)
so f is a FREE-AXIS weighted reduction over the per-row-chunk exp tiles
(DVE multiply + row-sum per chunk) instead of 16 tensor-engine matvecs
per batch.  PE work per batch is 3 spread transposes (compute engines can
only address partitions at multiples of 32, so stat columns are spread at
free offsets 0/32/64/96 first) + 4 rank-2 outer-product matmuls against a
[2, C] stacked stats tile that a tiny SBUF->SBUF DMA builds (DMA has no
partition-base restriction).

Sharding: data-parallel over batch across 8 NeuronCores (4 batches/core).
Each core streams its 32 MiB shard once (input DMAs alone on the sync
HWDGE queue) and writes the 32 MiB broadcast output (output DMAs alone on
the gpsimd SWDGE queue) so neither stream head-of-line blocks the other.
The per-batch stats chain (~25us of cross-engine latency) is software-
pipelined: chain(k) is emitted before red(k+1), so its stalls overlap the
next batch's input streaming and the previous batch's output drain.
"""

import os
import sys
from contextlib import ExitStack

import numpy as np

for _p in (
    "/opt/trn_rl_repo",
    "/root/.axon_site",
    "/root/.axon_site/_ro/trn_rl_repo",
    "/root/.axon_site/_ro/pypackages",
):
    if os.path.isdir(_p) and _p not in sys.path:
        sys.path.append(_p)

import concourse.bass as bass  # noqa: E402
import concourse.tile as tile  # noqa: E402
from concourse import bacc, masks, mybir  # noqa: E402
from concourse.bass_utils import run_bass_kernel_spmd  # noqa: E402

F32 = mybir.dt.float32
AX = mybir.AxisListType.X
AF = mybir.ActivationFunctionType
MUL = mybir.AluOpType.mult
ADD = mybir.AluOpType.add

B, C, H, W = 32, 512, 64, 64
S = H * W                # 4096 spatial positions
NCORES = 8
BL = B // NCORES         # 4 batches per core
NCH = C // 128           # 4 channel chunks of 128


def _emit(tc, out, x, alpha, beta):
    nc = tc.nc
    with ExitStack() as ctx:
        const = ctx.enter_context(tc.tile_pool(name="const", bufs=1))
        xpool = ctx.enter_context(tc.tile_pool(name="xin", bufs=5))
        bpool = ctx.enter_context(tc.tile_pool(name="bcast", bufs=3))
        epool = ctx.enter_context(tc.tile_pool(name="expt", bufs=8))
        vpool = ctx.enter_context(tc.tile_pool(name="vrow", bufs=2))
        wpool = ctx.enter_context(tc.tile_pool(name="wbc", bufs=2))
        small = ctx.enter_context(tc.tile_pool(name="small", bufs=3))
        fpool = ctx.enter_context(tc.tile_pool(name="fcols", bufs=4))
        zpsum = ctx.enter_context(tc.tile_pool(name="zp", bufs=2, space="PSUM"))
        tpsum = ctx.enter_context(tc.tile_pool(name="tp", bufs=2, space="PSUM"))

        xts = {}

        def load_batch(b):
            ts = []
            for cc in range(NCH):
                xt = xpool.tile([128, S], F32)
                nc.sync.dma_start(xt[:], x[b, cc * 128:(cc + 1) * 128, :])
                ts.append(xt)
            xts[b] = ts

        # batch 0's input DMAs lead the sync queue so streaming starts
        # immediately
        load_batch(0)

        ident = const.tile([128, 128], F32)
        masks.make_identity(nc, ident[:])
        zeros = const.tile([128, S], F32)
        nc.vector.memset(zeros[:], 0.0)
        # scratch sink for the scalar-engine pooling sums (never read)
        trash = const.tile([128, S], mybir.dt.bfloat16)
        # scratch for the DVE multiply-reduce f computation (never read)
        scr = const.tile([128, C], F32)
        ab = const.tile([1, 2], F32)
        nc.scalar.dma_start(ab[0:1, 0:1], alpha[:])
        nc.scalar.dma_start(ab[0:1, 1:2], beta[:])
        ab_bc = const.tile([128, 2], F32)
        nc.gpsimd.partition_broadcast(ab_bc[:], ab[0:1, :])

        ffs = {}

        # compute engines may only address partitions at multiples of 32, so
        # stat columns are spread at free offsets 0/32/64/96 before the PE
        # transpose; the transposed rows then land on legal partition bases.
        SPREAD = 32 * (NCH - 1) + 1

        des = {}

        def red(b):
            # ---- pooling: d = max (DVE), esum (ACT accum) over spatial ----
            # max uses a halving tree: tensor_tensor max runs ~2x the
            # elem rate of a full-width reduce, so 3 halvings + a narrow
            # reduce beat one wide reduce.
            dS = small.tile([128, SPREAD], F32)
            eS = small.tile([128, SPREAD], F32)
            for t in range(NCH):
                xt = xts[b][t]
                nc.vector.reduce_max(dS[:, 32 * t:32 * t + 1], xt[:], axis=AX)
                nc.scalar.activation(
                    trash[:], xt[:], AF.Copy,
                    accum_out=eS[:, 32 * t:32 * t + 1],
                )
            des[b] = (dS, eS)

        def chain(b):
            dS, eS = des[b]
            # g = alpha*d + (beta/S)*esum ; esum scaled to mean in place
            g4 = small.tile([128, NCH], F32)
            gt = small.tile([128, NCH], F32)
            nc.vector.tensor_scalar_mul(g4[:], dS[:, 0:SPREAD:32],
                                        ab_bc[:, 0:1])
            nc.vector.tensor_scalar(gt[:], eS[:, 0:SPREAD:32], ab_bc[:, 1:2],
                                    1.0 / S, op0=MUL, op1=MUL)
            nc.vector.tensor_add(g4[:], g4[:], gt[:])
            nc.vector.tensor_scalar_mul(eS[:, 0:SPREAD:32],
                                        eS[:, 0:SPREAD:32], 1.0 / S)

            # ---- PE transposes; V = [d_row | e_row] on partition 0; then a
            # tiny SBUF->SBUF DMA restacks it as V2 = [d_row; e_row] on two
            # partitions (DMA has no partition-base restriction), halving the
            # z matmul count via a single k=2 contraction per chunk.
            tpd = tpsum.tile([SPREAD, 128], F32)
            nc.tensor.transpose(tpd[:], dS[:], ident[:])
            tpe = tpsum.tile([SPREAD, 128], F32)
            nc.tensor.transpose(tpe[:], eS[:], ident[:])
            V = vpool.tile([1, 2 * C], F32)
            for cc in range(NCH):
                nc.vector.tensor_copy(V[0:1, cc * 128:(cc + 1) * 128],
                                      tpd[32 * cc:32 * cc + 1, :])
                nc.vector.tensor_copy(V[0:1, C + cc * 128:C + (cc + 1) * 128],
                                      tpe[32 * cc:32 * cc + 1, :])
            V2 = vpool.tile([2, C], F32)
            nc.scalar.dma_start(V2[:], V[0:1, :])

            # ---- z rows per chunk (rank-2 matmul), E = exp(z-m), s = rowsum
            nm4 = small.tile([128, NCH], F32)   # -m per row
            ss4 = small.tile([128, NCH], F32)   # rowsum of exp
            ets = []
            for ic in range(NCH):
                zp = zpsum.tile([128, C], F32)
                nc.tensor.matmul(zp[:], V2[:, ic * 128:(ic + 1) * 128],
                                 V2[:], start=True, stop=True)
                nc.vector.reduce_max(nm4[:, ic:ic + 1], zp[:], axis=AX,
                                     negate=True)
                et = epool.tile([128, C], F32)
                nc.scalar.activation(et[:], zp[:], AF.Exp,
                                     bias=nm4[:, ic:ic + 1], scale=1.0,
                                     accum_out=ss4[:, ic:ic + 1])
                ets.append(et)

            # ---- w = g * e^{-m} / s  (columns), then to broadcast row form
            rs4 = small.tile([128, NCH], F32)
            nc.vector.reciprocal(rs4[:], ss4[:])
            emn4 = small.tile([128, NCH], F32)  # e^{-m}
            nc.scalar.activation(emn4[:], nm4[:], AF.Exp)
            emx4 = small.tile([128, NCH], F32)  # e^{+m}
            nc.scalar.activation(emx4[:], nm4[:], AF.Exp, scale=-1.0)
            w4 = small.tile([128, SPREAD], F32)
            nc.vector.tensor_mul(w4[:, 0:SPREAD:32], g4[:], emn4[:])
            nc.vector.tensor_mul(w4[:, 0:SPREAD:32], w4[:, 0:SPREAD:32],
                                 rs4[:])
            tw = tpsum.tile([SPREAD, 128], F32)
            nc.tensor.transpose(tw[:], w4[:], ident[:])
            wrow = vpool.tile([1, C], F32)
            for cc in range(NCH):
                nc.vector.tensor_copy(wrow[0:1, cc * 128:(cc + 1) * 128],
                                      tw[32 * cc:32 * cc + 1, :])
            wbc = wpool.tile([128, C], F32)
            nc.gpsimd.partition_broadcast(wbc[:], wrow[0:1, :])

            # ---- f columns: f[jc] = e^{m} * sum_i w_i * E_jc[:, i] ----
            # (multiply on DVE; row-sum via the ACT accumulator)
            ff = fpool.tile([128, NCH], F32)
            for jc in range(NCH):
                nc.vector.tensor_mul(scr[:], ets[jc][:], wbc[:])
                nc.vector.reduce_sum(ff[:, jc:jc + 1], scr[:], axis=AX)
                nc.vector.tensor_mul(ff[:, jc:jc + 1], ff[:, jc:jc + 1],
                                     emx4[:, jc:jc + 1])
            ffs[b] = ff

        def emit_out(b):
            ff = ffs[b]
            for jc in range(NCH):
                bc = bpool.tile([128, S], F32)
                if jc % 2 == 0:
                    nc.vector.tensor_scalar_add(bc[:], zeros[:],
                                                ff[:, jc:jc + 1])
                else:
                    nc.scalar.activation(bc[:], zeros[:], AF.Identity,
                                         bias=ff[:, jc:jc + 1], scale=1.0)
                nc.gpsimd.dma_start(
                    out[b, jc * 128:(jc + 1) * 128, :], bc[:])

        # software pipeline: batch k's chain (long cross-engine latency)
        # is emitted BEFORE batch k+1's reduces on every engine, so the
        # chain stalls overlap the next batch's input streaming and the
        # previous batch's output drain.
        red(0)
        load_batch(1)
        chain(0)
        emit_out(0)
        red(1)
        load_batch(2)
        chain(1)
        emit_out(1)
        red(2)
        load_batch(3)
        chain(2)
        emit_out(2)
        red(3)
        chain(3)
        emit_out(3)


_CACHE = {}
LAST_RESULTS = None


def _build():
    nc = bacc.Bacc("TRN2", target_bir_lowering=False, debug=False,
                   enable_asserts=False, num_devices=NCORES)
    x = nc.dram_tensor("x", [BL, C, S], F32, kind="ExternalInput").ap()
    alpha = nc.dram_tensor("alpha", [1], F32, kind="ExternalInput").ap()
    beta = nc.dram_tensor("beta", [1], F32, kind="ExternalInput").ap()
    out = nc.dram_tensor("out", [BL, C, S], F32, kind="ExternalOutput").ap()
    with tile.TileContext(nc) as tc:
        _emit(tc, out, x, alpha, beta)
    nc.compile()
    return nc


def kernel(x, alpha, beta, _trace=False):
    global LAST_RESULTS
    if "nc" not in _CACHE:
        _CACHE["nc"] = _build()
    nc = _CACHE["nc"]

    xs = np.ascontiguousarray(np.asarray(x, dtype=np.float32).reshape(B, C, S))
    a = np.ascontiguousarray(np.asarray(alpha, dtype=np.float32).reshape(1))
    bt = np.ascontiguousarray(np.asarray(beta, dtype=np.float32).reshape(1))
    in_maps = [
        {"x": xs[k * BL:(k + 1) * BL], "alpha": a, "beta": bt}
        for k in range(NCORES)
    ]
    res = run_bass_kernel_spmd(nc, in_maps, list(range(NCORES)), trace=_trace)
    LAST_RESULTS = res
    full = np.concatenate(
        [np.asarray(res.results[k]["out"]) for k in range(NCORES)], axis=0
    )
    return full.reshape(B, C, H, W).astype(np.float32, copy=False)
